# revision 17
# baseline (speedup 1.0000x reference)
"""Trainium2 Bass kernel for nn_CustomMetalPKA_GNN (gnn_message_passing).

Distribution: node-sharded GCN message passing across 8 NeuronCores.
Each core owns a contiguous block of 1280 node rows (10 windows of 128).
Edges are assigned to the core owning their destination node, sorted by
destination, and processed as 128-edge tiles:
  gather(src rows via dma_gather) -> one-hot(dst) scatter-matmul into PSUM.
The symmetric GCN norm is factorized: tables hold dinv[s] * row, the
window epilogue applies dinv[d]. Self-loops are appended as ordinary
(d, d) edges. Layer boundary uses one AllGather of the per-core table
rows. The tiny metal/transformer tail is reduced to an [8, 512] summary
(3 ligand-block sums + 3 prediction rows) via a mask matmul, and
finished on host (0.006% of FLOPs).
"""

import os
import sys

for _p in ("/opt/trn_rl_repo", "/root/.axon_site/_ro/trn_rl_repo"):
    if os.path.isdir(_p) and _p not in sys.path:
        sys.path.insert(0, _p)

import numpy as np

import concourse.bacc as bacc
import concourse.tile as tile
from concourse import bass, mybir
from concourse.bass_utils import run_bass_kernel_spmd
from concourse.masks import make_identity

# Problem shapes (hardcoded per spec)
N = 9999
E = 160000
NODE_D = 256
HID = 512
MAX_LIG = 3
APL = N // MAX_LIG  # 3333

NCORES = 8
P = 128
WPC = 10                 # windows per core
NPC = WPC * P            # 1280 nodes per core
NPAD = NCORES * NPC      # 10240
NW = NCORES * WPC        # 80 global windows

FP = mybir.dt.float32
I16 = mybir.dt.int16
GCHUNK = 6  # gather tiles per dma_gather call (SWDGE desc ring <= 1024)

_RUN_CACHE = {}


# ----------------------------------------------------------------------------
# Host-side graph preprocessing (index/structure only)
# ----------------------------------------------------------------------------

def _prep(x, edge_index, pred_pos):
    src = np.asarray(edge_index[0], dtype=np.int64)
    dst = np.asarray(edge_index[1], dtype=np.int64)
    pred_pos = np.asarray(pred_pos, dtype=np.int64)

    deg = np.bincount(dst, minlength=N).astype(np.float32) + 1.0

    order = np.argsort(dst, kind="stable")
    s_s = src[order]
    d_s = dst[order]

    # per (core, window) edge slices
    bounds = np.empty((NCORES, WPC, 2), np.int64)
    for c in range(NCORES):
        for w in range(WPC):
            lo = c * NPC + w * P
            hi = min(lo + P, N)
            if lo >= N:
                lo = hi = N  # empty
            bounds[c, w, 0] = np.searchsorted(d_s, lo, side="left")
            bounds[c, w, 1] = np.searchsorted(d_s, hi, side="left")

    # tiles per window: edges + self-loops (for real nodes), padded to 128
    T = []
    for w in range(WPC):
        mx = 1
        for c in range(NCORES):
            lo = c * NPC + w * P
            nself = max(0, min(lo + P, N) - lo)
            cnt = int(bounds[c, w, 1] - bounds[c, w, 0]) + nself
            mx = max(mx, (cnt + P - 1) // P)
        T.append(mx)
    Tsum = sum(T)
    CTOT = 8 * Tsum  # int16 index columns

    # per-core arrays
    per_core = []
    for c in range(NCORES):
        gidx = np.zeros((P, CTOT), np.int16)
        doff = np.full((P, Tsum), -1.0, np.float32)
        off = 0
        for w in range(WPC):
            lo, hi = int(bounds[c, w, 0]), int(bounds[c, w, 1])
            base = c * NPC + w * P
            nself = max(0, min(base + P, N) - base)
            e_src = np.concatenate([s_s[lo:hi], np.arange(base, base + nself)])
            e_off = np.concatenate([d_s[lo:hi] - base, np.arange(nself)])
            n = e_src.shape[0]
            cap = T[w] * P
            srcs = np.zeros(cap, np.int64)
            offs = np.full(cap, -1.0, np.float32)
            srcs[:n] = e_src
            offs[:n] = e_off.astype(np.float32)
            # pack indices: entry i -> gidx[i % 16, colbase + i // 16]
            # (replicated to all 8 Q7-core stripes of 16 partitions below)
            colbase = 8 * off
            ii = np.arange(cap)
            gidx[ii % 16, colbase + ii // 16] = srcs.astype(np.int16)
            doff[:, off:off + T[w]] = offs.reshape(T[w], P).T
            off += T[w]
        gidx[16:] = np.tile(gidx[:16], (7, 1))

        # deg layout [P, 80]: degT[p, g] = deg[node g*128+p] (1.0 for pads)
        degT = np.ones((P, NW), np.float32)
        flat = np.ones(NPAD, np.float32)
        flat[:N] = deg
        degT[:, :] = flat.reshape(NW, P).T

        # tail masks [P, 8 * WPC]
        tmask = np.zeros((P, 8 * WPC), np.float32)
        for w in range(WPC):
            base = c * NPC + w * P
            nodes = base + np.arange(P)
            real = nodes < N
            for b in range(MAX_LIG):
                sel = real & (nodes >= b * APL) & (nodes < (b + 1) * APL)
                tmask[sel, 8 * w + b] = 1.0
            for i in range(MAX_LIG):
                sel = nodes == pred_pos[i]
                tmask[sel, 8 * w + 3 + i] = 1.0
        per_core.append(dict(gidx=gidx, doff=doff, degT=degT, tmask=tmask))

    x_pad = np.zeros((NPAD, NODE_D), np.float32)
    x_pad[:N] = np.asarray(x, np.float32)

    iota = np.tile(np.arange(P, dtype=np.float32)[None, :], (P, 1))

    meta = dict(T=T, Tsum=Tsum, CTOT=CTOT)
    return meta, per_core, x_pad, iota


# ----------------------------------------------------------------------------
# Device program
# ----------------------------------------------------------------------------

def _build(meta, sim1=False):
    T = meta["T"]
    Tsum = meta["Tsum"]
    CTOT = meta["CTOT"]

    NWT = NW + WPC  # deg columns: 80 global + 10 own-window copies

    nc = bacc.Bacc("TRN2", target_bir_lowering=False, debug=False,
                   num_devices=1 if sim1 else NCORES)

    # inputs
    d_x = nc.declare_dram_parameter("x_pad", [NPAD, NODE_D], FP, isOutput=False)
    d_gidx = nc.declare_dram_parameter("gidx", [P, CTOT], I16, isOutput=False)
    d_doff = nc.declare_dram_parameter("doff", [P, Tsum], FP, isOutput=False)
    d_degT = nc.declare_dram_parameter("degT", [P, NWT], FP, isOutput=False)
    d_tmask = nc.declare_dram_parameter("tmask", [P, 8 * WPC], FP, isOutput=False)
    d_iota = nc.declare_dram_parameter("iota", [P, P], FP, isOutput=False)
    d_g1 = nc.declare_dram_parameter("g1_w", [NODE_D, HID], FP, isOutput=False)
    d_g2 = nc.declare_dram_parameter("g2_w", [HID, HID], FP, isOutput=False)
    d_lp = nc.declare_dram_parameter("lp_w", [HID, HID], FP, isOutput=False)
    d_b1 = nc.declare_dram_parameter("b1_rep", [P, HID], FP, isOutput=False)
    d_b2 = nc.declare_dram_parameter("b2_rep", [P, HID], FP, isOutput=False)
    d_blp = nc.declare_dram_parameter("blp_rep", [P, HID], FP, isOutput=False)
    # output
    d_tail = nc.declare_dram_parameter("out_tail", [8, HID], FP, isOutput=True)

    # internal dram
    table1 = nc.dram_tensor("table1", [NPAD, NODE_D], FP)
    local_rows = nc.dram_tensor("local_rows", [NPC, HID], FP)
    table2 = nc.dram_tensor("table2", [NPAD, HID], FP, addr_space="Shared")

    mm = mybir.AluOpType
    act = mybir.ActivationFunctionType

    with tile.TileContext(nc) as tc:
        with (
            tc.tile_pool(name="const", bufs=1) as cpool,
            tc.tile_pool(name="oh", bufs=4) as ohpool,
            tc.tile_pool(name="work", bufs=3) as wpool,
            tc.tile_pool(name="hT", bufs=2) as hTpool,
            tc.tile_pool(name="ps_sc", bufs=2, space="PSUM") as ps_sc,
            tc.tile_pool(name="ps_tr", bufs=2, space="PSUM") as ps_tr,
            tc.tile_pool(name="ps_mm", bufs=2, space="PSUM") as ps_mm,
            tc.tile_pool(name="ps_tail", bufs=1, space="PSUM") as ps_tail,
        ):
            # ---- constants in ----
            t_idx = cpool.tile([P, CTOT], I16)
            nc.sync.dma_start(t_idx[:], d_gidx[:])
            t_doff = cpool.tile([P, Tsum], FP)
            nc.sync.dma_start(t_doff[:], d_doff[:])
            t_iota = cpool.tile([P, P], FP)
            nc.sync.dma_start(t_iota[:], d_iota[:])
            t_tmask = cpool.tile([P, 8 * WPC], FP)
            nc.sync.dma_start(t_tmask[:], d_tmask[:])
            t_g1 = cpool.tile([P, NODE_D // P, HID], FP)
            nc.sync.dma_start(t_g1[:], d_g1.rearrange("(a p) n -> p a n", p=P))
            t_g2 = cpool.tile([P, HID // P, HID], FP)
            nc.sync.dma_start(t_g2[:], d_g2.rearrange("(a p) n -> p a n", p=P))
            t_lp = cpool.tile([P, HID // P, HID], FP)
            nc.sync.dma_start(t_lp[:], d_lp.rearrange("(a p) n -> p a n", p=P))
            t_b1 = cpool.tile([P, HID], FP)
            nc.sync.dma_start(t_b1[:], d_b1[:])
            t_b2 = cpool.tile([P, HID], FP)
            nc.sync.dma_start(t_b2[:], d_b2[:])
            t_blp = cpool.tile([P, HID], FP)
            nc.sync.dma_start(t_blp[:], d_blp[:])
            t_ident = cpool.tile([P, P], FP)
            make_identity(nc, t_ident[:])

            # dinvT = 1/sqrt(deg)  [P, 90]: cols 0..79 global, 80..89 own
            t_deg = cpool.tile([P, NWT], FP)
            nc.sync.dma_start(t_deg[:], d_degT[:])
            t_sq = cpool.tile([P, NWT], FP)
            nc.scalar.activation(t_sq[:], t_deg[:], act.Sqrt)
            t_dinv = cpool.tile([P, NWT], FP)
            nc.vector.reciprocal(t_dinv[:], t_sq[:])

            def onehot(w, t, off_w):
                oh = ohpool.tile([P, P], FP, tag="oh")
                col = off_w + t
                nc.vector.tensor_tensor(
                    out=oh[:],
                    in0=t_doff[:, col:col + 1].to_broadcast([P, P]),
                    in1=t_iota[:],
                    op=mm.is_equal)
                return oh

            # ---- table1 = dinv * x (all 80 windows, full-local) ----
            # per-core rank offset not needed: every core scales all rows.
            for g in range(NW):
                xt = wpool.tile([P, NODE_D], FP, tag="xt")
                nc.sync.dma_start(xt[:], d_x[g * P:(g + 1) * P, :])
                sc = wpool.tile([P, NODE_D], FP, tag="xsc")
                nc.vector.tensor_scalar_mul(sc[:], xt[:], t_dinv[:, g:g + 1])
                nc.sync.dma_start(table1[g * P:(g + 1) * P, :], sc[:])

            # ---- layer 1 + table2 rows ----
            off_w = 0
            g1ctx = tc.tile_pool(name="g1pool", bufs=2)
            g1pool = g1ctx.__enter__()
            for w in range(WPC):
                Tw = T[w]
                g1t = g1pool.tile([P, Tw, NODE_D], FP, tag="gather1")
                for t0 in range(0, Tw, GCHUNK):
                    t1 = min(t0 + GCHUNK, Tw)
                    nc.gpsimd.dma_gather(
                        out_ap=g1t[:, t0:t1, :],
                        in_ap=table1[:],
                        idxs_ap=t_idx[:, 8 * (off_w + t0): 8 * (off_w + t1)],
                        num_idxs=(t1 - t0) * P,
                        num_idxs_reg=(t1 - t0) * P,
                        elem_size=NODE_D)
                psx = ps_sc.tile([P, NODE_D], FP, tag="psc")
                for t in range(Tw):
                    oh = onehot(w, t, off_w)
                    nc.tensor.matmul(psx[:], oh[:], g1t[:, t, :],
                                     start=(t == 0), stop=(t == Tw - 1))
                sx = wpool.tile([P, NODE_D], FP, tag="sx")
                nc.vector.tensor_copy(sx[:], psx[:])
                # transpose 2 chunks -> lhsT
                sxT = []
                for k in range(NODE_D // P):
                    pT = ps_tr.tile([P, P], FP, tag="ptr")
                    nc.tensor.transpose(pT[:], sx[:, k * P:(k + 1) * P], t_ident[:])
                    sT = hTpool.tile([P, P], FP, tag="sxT")
                    nc.vector.tensor_copy(sT[:], pT[:])
                    sxT.append(sT)
                ph = ps_mm.tile([P, HID], FP, tag="pmm")
                for k in range(NODE_D // P):
                    nc.tensor.matmul(ph[:], sxT[k][:], t_g1[:, k, :],
                                     start=(k == 0), stop=(k == NODE_D // P - 1))
                # h = relu(ph * dinv_own[w] + b1)
                hw_ = wpool.tile([P, HID], FP, tag="hwin")
                nc.vector.scalar_tensor_tensor(
                    out=hw_[:], in0=ph[:], scalar=t_dinv[:, NW + w:NW + w + 1],
                    in1=t_b1[:], op0=mm.mult, op1=mm.add)
                nc.scalar.activation(hw_[:], hw_[:], act.Relu)
                # table2 rows = dinv_own[w] * (h @ g2_w)
                hT = []
                for k in range(HID // P):
                    pT = ps_tr.tile([P, P], FP, tag="ptr")
                    nc.tensor.transpose(pT[:], hw_[:, k * P:(k + 1) * P], t_ident[:])
                    sT = hTpool.tile([P, P], FP, tag="hT")
                    nc.vector.tensor_copy(sT[:], pT[:])
                    hT.append(sT)
                pt2 = ps_mm.tile([P, HID], FP, tag="pmm")
                for k in range(HID // P):
                    nc.tensor.matmul(pt2[:], hT[k][:], t_g2[:, k, :],
                                     start=(k == 0), stop=(k == HID // P - 1))
                t2 = wpool.tile([P, HID], FP, tag="t2")
                nc.vector.tensor_scalar_mul(t2[:], pt2[:],
                                            t_dinv[:, NW + w:NW + w + 1])
                nc.sync.dma_start(local_rows[w * P:(w + 1) * P, :], t2[:])
                off_w += Tw

            g1ctx.__exit__(None, None, None)

            # ---- exchange ----
            if sim1:
                # timeline-sim stand-in: local copy only (no collectives in sim)
                for g in range(WPC):
                    tt = wpool.tile([P, HID], FP, tag="t2")
                    nc.sync.dma_start(tt[:], local_rows[g * P:(g + 1) * P, :])
                    nc.sync.dma_start(table2[g * P:(g + 1) * P, :], tt[:])
            else:
                nc.gpsimd.collective_compute(
                    "AllGather", mm.bypass,
                    replica_groups=[list(range(NCORES))],
                    ins=[local_rows[:]],
                    outs=[table2[:]])

            # ---- layer 2 + proj + tail ----
            ptail = ps_tail.tile([8, HID], FP)
            off_w = 0
            g2ctx = tc.tile_pool(name="g2pool", bufs=2)
            g2pool = g2ctx.__enter__()
            for w in range(WPC):
                Tw = T[w]
                g2t = g2pool.tile([P, Tw, HID], FP, tag="gather2")
                for t0 in range(0, Tw, GCHUNK):
                    t1 = min(t0 + GCHUNK, Tw)
                    nc.gpsimd.dma_gather(
                        out_ap=g2t[:, t0:t1, :],
                        in_ap=table2[:],
                        idxs_ap=t_idx[:, 8 * (off_w + t0): 8 * (off_w + t1)],
                        num_idxs=(t1 - t0) * P,
                        num_idxs_reg=(t1 - t0) * P,
                        elem_size=HID)
                ps2 = ps_sc.tile([P, HID], FP, tag="psc")
                for t in range(Tw):
                    oh = onehot(w, t, off_w)
                    nc.tensor.matmul(ps2[:], oh[:], g2t[:, t, :],
                                     start=(t == 0), stop=(t == Tw - 1))
                h2 = wpool.tile([P, HID], FP, tag="h2win")
                nc.vector.scalar_tensor_tensor(
                    out=h2[:], in0=ps2[:], scalar=t_dinv[:, NW + w:NW + w + 1],
                    in1=t_b2[:], op0=mm.mult, op1=mm.add)
                nc.scalar.activation(h2[:], h2[:], act.Relu)
                h2T = []
                for k in range(HID // P):
                    pT = ps_tr.tile([P, P], FP, tag="ptr")
                    nc.tensor.transpose(pT[:], h2[:, k * P:(k + 1) * P], t_ident[:])
                    sT = hTpool.tile([P, P], FP, tag="hT")
                    nc.vector.tensor_copy(sT[:], pT[:])
                    h2T.append(sT)
                ph1 = ps_mm.tile([P, HID], FP, tag="pmm")
                for k in range(HID // P):
                    nc.tensor.matmul(ph1[:], h2T[k][:], t_lp[:, k, :],
                                     start=(k == 0), stop=(k == HID // P - 1))
                h1 = wpool.tile([P, HID], FP, tag="h1win")
                nc.vector.tensor_add(h1[:], ph1[:], t_blp[:])
                nc.scalar.activation(h1[:], h1[:], act.Relu)
                nc.tensor.matmul(ptail[:], t_tmask[:, 8 * w:8 * w + 8], h1[:],
                                 start=(w == 0), stop=(w == WPC - 1))
                off_w += Tw

            g2ctx.__exit__(None, None, None)

            t_tail = wpool.tile([8, HID], FP, tag="tailout")
            nc.vector.tensor_copy(t_tail[:], ptail[:])
            nc.sync.dma_start(d_tail[:], t_tail[:])

    nc.compile()
    return nc


# ----------------------------------------------------------------------------
# Host tail (metal branch + gates + 4-node TransformerConv + MLP head)
# ----------------------------------------------------------------------------

def _host_tail(tail, pred_pos, metal_id, metal_emb_table, mp_w, mp_b,
               gate_w1, gate_b1, gate_w2, gate_b2,
               tq_w, tq_b, tk_w, tk_b, tv_w, tv_b, tskip_w, tskip_b,
               pr_w1, pr_b1, pr_w2, pr_b2):
    f = np.float32
    pred_pos = np.asarray(pred_pos, np.int64)
    blocksum = tail[:3].astype(f)
    predrow = tail[3:6].astype(f)
    HEADS, HD = 8, HID // 8

    backbones = []
    for i in range(MAX_LIG):
        b = int(pred_pos[i]) // APL
        backbones.append((blocksum[b] - predrow[i]) / f(APL - 1))

    metal_node = np.maximum(
        np.asarray(metal_emb_table, f)[np.asarray(metal_id, np.int64)] @
        np.asarray(mp_w, f) + np.asarray(mp_b, f), 0)

    def tconv(hm, es, ed):
        n = hm.shape[0]
        q = (hm @ np.asarray(tq_w, f) + np.asarray(tq_b, f)).reshape(n, HEADS, HD)
        k = (hm @ np.asarray(tk_w, f) + np.asarray(tk_b, f)).reshape(n, HEADS, HD)
        v = (hm @ np.asarray(tv_w, f) + np.asarray(tv_b, f)).reshape(n, HEADS, HD)
        kj = k[es]
        vj = v[es]
        alpha = (q[ed] * kj).sum(-1) / np.sqrt(f(HD))
        amax = np.full((n, HEADS), -np.inf, f)
        np.maximum.at(amax, ed, alpha)
        ae = np.exp(alpha - amax[ed])
        den = np.zeros((n, HEADS), f)
        np.add.at(den, ed, ae)
        att = ae / den[ed]
        out = np.zeros((n, HEADS, HD), f)
        np.add.at(out, ed, vj * att[:, :, None])
        return out.reshape(n, HID) + hm @ np.asarray(tskip_w, f) + np.asarray(tskip_b, f)

    preds = []
    for n_lig in range(MAX_LIG, 0, -1):
        rows = []
        for i in range(n_lig):
            hb = backbones[i]
            g = 1.0 / (1.0 + np.exp(-(np.tanh(hb @ np.asarray(gate_w1, f) +
                                              np.asarray(gate_b1, f)) @
                                      np.asarray(gate_w2, f) +
                                      np.asarray(gate_b2, f))))
            rows.append(predrow[i] + g[0] * hb)
        hm = np.concatenate([metal_node, np.stack(rows)], 0).astype(f)
        es, ed = [], []
        for l in range(1, n_lig + 1):
            es += [0, l]
            ed += [l, 0]
        h3 = tconv(hm, np.array(es), np.array(ed))
        V = h3.mean(0)
        preds.append((V @ np.asarray(pr_w1, f) + np.asarray(pr_b1, f)) @
                     np.asarray(pr_w2, f) + np.asarray(pr_b2, f))
    return np.concatenate(preds).astype(np.float32)


# ----------------------------------------------------------------------------
# Entry point
# ----------------------------------------------------------------------------

def kernel(**inputs):
    x = np.asarray(inputs["x"], np.float32)
    edge_index = np.asarray(inputs["edge_index"])
    pred_pos = np.asarray(inputs["pred_pos"])

    meta, per_core, x_pad, iota = _prep(x, edge_index, pred_pos)

    b1_rep = np.tile(np.asarray(inputs["g1_b"], np.float32)[None, :], (P, 1))
    b2_rep = np.tile(np.asarray(inputs["g2_b"], np.float32)[None, :], (P, 1))
    blp_rep = np.tile(np.asarray(inputs["lp_b"], np.float32)[None, :], (P, 1))

    in_maps = []
    for c in range(NCORES):
        pc = per_core[c]
        # append own windows' deg columns at [80..90) for rank-local epilogues
        deg_own = pc["degT"][:, c * WPC:(c + 1) * WPC]
        degT = np.concatenate([pc["degT"], deg_own], axis=1)
        in_maps.append(dict(
            x_pad=x_pad,
            gidx=pc["gidx"],
            doff=pc["doff"],
            degT=degT,
            tmask=pc["tmask"],
            iota=iota,
            g1_w=np.asarray(inputs["g1_w"], np.float32),
            g2_w=np.asarray(inputs["g2_w"], np.float32),
            lp_w=np.asarray(inputs["lp_w"], np.float32),
            b1_rep=b1_rep, b2_rep=b2_rep, blp_rep=blp_rep,
        ))

    key = (meta["Tsum"], tuple(meta["T"]))
    nc = _RUN_CACHE.get(key)
    if nc is None:
        nc = _build(meta)
        _RUN_CACHE[key] = nc

    res = run_bass_kernel_spmd(nc, in_maps, list(range(NCORES)))
    tail = np.zeros((8, HID), np.float32)
    for c in range(NCORES):
        tail += res.results[c]["out_tail"]

    return _host_tail(
        tail, pred_pos, inputs["metal_id"], inputs["metal_emb_table"],
        inputs["mp_w"], inputs["mp_b"],
        inputs["gate_w1"], inputs["gate_b1"], inputs["gate_w2"], inputs["gate_b2"],
        inputs["tq_w"], inputs["tq_b"], inputs["tk_w"], inputs["tk_b"],
        inputs["tv_w"], inputs["tv_b"], inputs["tskip_w"], inputs["tskip_b"],
        inputs["pr_w1"], inputs["pr_b1"], inputs["pr_w2"], inputs["pr_b2"])
